# revision 3
# baseline (speedup 1.0000x reference)
"""CenterLoss Trainium2 kernel (8 NeuronCores, data-parallel over batch).

Math: the reference builds the full [N, C] masked distance matrix, but only
the labeled entry of each row survives the mask, so

    loss = ( sum_i ||x_i - centers[labels_i]||^2  +  N*(C-1)*CLAMP_MIN ) / N

(the second term is the clamp applied to the zeroed-out entries).

Layout strategy: host transposes x and centers to d-major ([D, *]) and wraps
labels into the 16-partition-interleaved int16 layout the gpsimd gather
expects. Per core the kernel then needs no on-device transposes:
  - DMA xT shard [128, 2048] and centersT [128, 1024] into SBUF
  - ap_gather (gpsimd DSP) gathers centersT columns by label -> g [128, 2048]
  - DVE subtract, ACT square-with-row-accumulate, PE ones-matmul reduction
Host sums the 8 per-core partials.
"""

import numpy as np

import concourse.bacc as bacc
import concourse.tile as tile
from concourse import bass, mybir
from concourse.bass_utils import run_bass_kernel_spmd

N, C, D = 16384, 1024, 128
N_CORES = 8
NS = N // N_CORES  # 2048 samples per core
P = 128
CLAMP_MIN = 1e-12

_cache = {}


def build_nc(n_chunk=4, act_split=True):
    assert NS % n_chunk == 0
    spc = NS // n_chunk  # samples per chunk

    nc = bacc.Bacc()
    xt = nc.declare_dram_parameter("xt", [D, NS], mybir.dt.float32, isOutput=False)
    ct = nc.declare_dram_parameter("ct", [D, C], mybir.dt.float32, isOutput=False)
    labels = nc.declare_dram_parameter(
        "labels", [P, NS // 16], mybir.dt.int16, isOutput=False
    )
    out = nc.declare_dram_parameter("out", [1, 1], mybir.dt.float32, isOutput=True)

    with tile.TileContext(nc) as tc:
        with (
            tc.tile_pool(name="data", bufs=1) as data,
            tc.tile_pool(name="small", bufs=1) as small,
            tc.tile_pool(name="psum", bufs=1, space="PSUM") as psump,
        ):
            x_sb = data.tile([P, NS], mybir.dt.float32)
            g_sb = data.tile([P, NS], mybir.dt.float32)
            d_sb = data.tile([P, NS], mybir.dt.float32)
            c_sb = data.tile([P, C], mybir.dt.float32)
            i_sb = small.tile([P, NS // 16], mybir.dt.int16)
            acc = small.tile([P, n_chunk], mybir.dt.float32)
            ones = small.tile([P, 1], mybir.dt.float32)

            nc.vector.memset(ones[:], 1.0)
            nc.sync.dma_start(out=i_sb[:], in_=labels[:, :])
            nc.sync.dma_start(out=c_sb[:], in_=ct[:, :])
            for k in range(n_chunk):
                ss = slice(k * spc, (k + 1) * spc)
                nc.sync.dma_start(out=x_sb[:, ss], in_=xt[:, ss])
                nc.gpsimd.ap_gather(
                    out_ap=g_sb[:, ss],
                    in_ap=c_sb[:],
                    idxs_ap=i_sb[:, k * (spc // 16) : (k + 1) * (spc // 16)],
                    channels=P,
                    num_elems=C,
                    d=1,
                    num_idxs=spc,
                )
                nc.vector.tensor_tensor(
                    out=d_sb[:, ss],
                    in0=x_sb[:, ss],
                    in1=g_sb[:, ss],
                    op=mybir.AluOpType.subtract,
                )
                if act_split:
                    nc.scalar.activation(
                        out=d_sb[:, ss],
                        in_=d_sb[:, ss],
                        func=mybir.ActivationFunctionType.Square,
                        accum_out=acc[:, k : k + 1],
                    )
                else:
                    nc.vector.tensor_tensor_reduce(
                        out=d_sb[:, ss],
                        in0=d_sb[:, ss],
                        in1=d_sb[:, ss],
                        scale=1.0,
                        scalar=0.0,
                        op0=mybir.AluOpType.mult,
                        op1=mybir.AluOpType.add,
                        accum_out=acc[:, k : k + 1],
                    )
            psum = psump.tile([1, n_chunk], mybir.dt.float32)
            nc.tensor.matmul(
                out=psum[:, :], lhsT=ones[:], rhs=acc[:], start=True, stop=True
            )
            res = small.tile([1, 1], mybir.dt.float32)
            nc.vector.reduce_sum(
                out=res[:1, :1], in_=psum[:1, :], axis=mybir.AxisListType.X
            )
            nc.sync.dma_start(out=out[:, :], in_=res[:1, :1])
    nc.compile()
    return nc


def _get_nc():
    if "nc" not in _cache:
        _cache["nc"] = build_nc()
    return _cache["nc"]


def wrap_labels(labels_shard):
    """[NS] int -> [128, NS//16] int16 wrapped per 16-partition group,
    replicated across the 8 gpsimd cores."""
    w = labels_shard.reshape(NS // 16, 16).T.astype(np.int16)  # [16, NS//16]
    return np.ascontiguousarray(np.tile(w, (8, 1)))


def make_in_maps(x, centers, labels):
    x = np.asarray(x, dtype=np.float32)
    centers = np.asarray(centers, dtype=np.float32)
    labels = np.asarray(labels)
    xt = np.ascontiguousarray(x.T)  # [D, N]
    ct = np.ascontiguousarray(centers.T)  # [D, C]
    in_maps = []
    for c in range(N_CORES):
        sl = slice(c * NS, (c + 1) * NS)
        in_maps.append(
            {
                "xt": np.ascontiguousarray(xt[:, sl]),
                "ct": ct,
                "labels": wrap_labels(labels[sl]),
            }
        )
    return in_maps


def finalize(results):
    total = sum(float(results[c]["out"][0, 0]) for c in range(N_CORES))
    total += N * (C - 1) * CLAMP_MIN
    return np.float32(total / N)


def kernel(x, centers, labels):
    nc = _get_nc()
    res = run_bass_kernel_spmd(
        nc, make_in_maps(x, centers, labels), core_ids=list(range(N_CORES))
    )
    return finalize(res.results)


# revision 5
# speedup vs baseline: 1.9344x; 1.9344x over previous
"""CenterLoss Trainium2 kernel (8 NeuronCores, data-parallel over batch).

Math: the reference builds the full [N, C] masked distance matrix, but only
the labeled entry of each row survives the mask, so

    loss = ( sum_i ||x_i - centers[labels_i]||^2  +  N*(C-1)*CLAMP_MIN ) / N

(the second term is the clamp applied to the zeroed-out entries).

Layout strategy: host transposes x and centers to d-major ([D, *]) and wraps
labels into the 16-partition-interleaved int16 layout the gpsimd gather
expects. Per core the kernel then needs no on-device transposes:
  - DMA xT shard [128, 2048] and centersT [128, 1024] into SBUF
  - ap_gather (gpsimd DSP) gathers centersT columns by label -> g [128, 2048]
  - DVE subtract, ACT square-with-row-accumulate, PE ones-matmul reduction
Host sums the 8 per-core partials.
"""

import numpy as np

import concourse.bacc as bacc
import concourse.tile as tile
from concourse import bass, mybir
from concourse.bass_utils import run_bass_kernel_spmd

N, C, D = 16384, 1024, 128
N_CORES = 8
NS = N // N_CORES  # 2048 samples per core
P = 128
CLAMP_MIN = 1e-12

_cache = {}


T = NS // P  # 16 gather tiles of 128 rows


def build_nc(n_chunk=4, act_split=True, n_xdma=4):
    """Native indirect-DMA gather version: 16 gathers of 128 rows each
    (no gpsimd custom-op library load), batched x DMAs, chunked DVE/ACT."""
    assert T % n_chunk == 0
    tpc = T // n_chunk  # tiles per chunk

    nc = bacc.Bacc()
    x = nc.declare_dram_parameter("x", [NS, D], mybir.dt.float32, isOutput=False)
    centers = nc.declare_dram_parameter(
        "centers", [C, D], mybir.dt.float32, isOutput=False
    )
    labels = nc.declare_dram_parameter("labels", [P, T], mybir.dt.int32, isOutput=False)
    out = nc.declare_dram_parameter("out", [1, 1], mybir.dt.float32, isOutput=True)

    x_t = x.rearrange("(t p) d -> p t d", p=P)  # [128, T, 128]

    with tile.TileContext(nc) as tc:
        with (
            tc.tile_pool(name="data", bufs=1) as data,
            tc.tile_pool(name="small", bufs=1) as small,
            tc.tile_pool(name="psum", bufs=1, space="PSUM") as psump,
        ):
            x_sb = data.tile([P, T, D], mybir.dt.float32)
            g_sb = data.tile([P, T, D], mybir.dt.float32)
            d_sb = data.tile([P, T, D], mybir.dt.float32)
            i_sb = small.tile([P, T], mybir.dt.int32)
            acc = small.tile([P, n_chunk], mybir.dt.float32)
            ones = small.tile([P, 1], mybir.dt.float32)

            nc.vector.memset(ones[:], 1.0)
            nc.sync.dma_start(out=i_sb[:], in_=labels[:, :])
            tpx = T // n_xdma
            for j in range(n_xdma):
                xs = slice(j * tpx, (j + 1) * tpx)
                nc.sync.dma_start(out=x_sb[:, xs, :], in_=x_t[:, xs, :])
            for t in range(T):
                nc.gpsimd.indirect_dma_start(
                    out=g_sb[:, t, :],
                    out_offset=None,
                    in_=centers[:],
                    in_offset=bass.IndirectOffsetOnAxis(ap=i_sb[:, t : t + 1], axis=0),
                )
            for k in range(n_chunk):
                ts = slice(k * tpc, (k + 1) * tpc)
                nc.vector.tensor_tensor(
                    out=d_sb[:, ts, :],
                    in0=x_sb[:, ts, :],
                    in1=g_sb[:, ts, :],
                    op=mybir.AluOpType.subtract,
                )
                if act_split:
                    nc.scalar.activation(
                        out=d_sb[:, ts, :],
                        in_=d_sb[:, ts, :],
                        func=mybir.ActivationFunctionType.Square,
                        accum_out=acc[:, k : k + 1],
                    )
                else:
                    nc.vector.tensor_tensor_reduce(
                        out=d_sb[:, ts, :],
                        in0=d_sb[:, ts, :],
                        in1=d_sb[:, ts, :],
                        scale=1.0,
                        scalar=0.0,
                        op0=mybir.AluOpType.mult,
                        op1=mybir.AluOpType.add,
                        accum_out=acc[:, k : k + 1],
                    )
            psum = psump.tile([1, n_chunk], mybir.dt.float32)
            nc.tensor.matmul(
                out=psum[:, :], lhsT=ones[:], rhs=acc[:], start=True, stop=True
            )
            res = small.tile([1, 1], mybir.dt.float32)
            nc.vector.reduce_sum(
                out=res[:1, :1], in_=psum[:1, :], axis=mybir.AxisListType.X
            )
            nc.sync.dma_start(out=out[:, :], in_=res[:1, :1])
    nc.compile()
    return nc


def _get_nc():
    if "nc" not in _cache:
        _cache["nc"] = build_nc()
    return _cache["nc"]


def prep_labels(labels_shard):
    """[NS] int -> [128, T] int32 with idx[p, t] = labels[t*128 + p]."""
    return np.ascontiguousarray(labels_shard.reshape(T, P).T.astype(np.int32))


def make_in_maps(x, centers, labels):
    x = np.ascontiguousarray(np.asarray(x, dtype=np.float32))
    centers = np.ascontiguousarray(np.asarray(centers, dtype=np.float32))
    labels = np.asarray(labels)
    in_maps = []
    for c in range(N_CORES):
        sl = slice(c * NS, (c + 1) * NS)
        in_maps.append(
            {
                "x": x[sl],
                "centers": centers,
                "labels": prep_labels(labels[sl]),
            }
        )
    return in_maps


def finalize(results):
    total = sum(float(results[c]["out"][0, 0]) for c in range(N_CORES))
    total += N * (C - 1) * CLAMP_MIN
    return np.float32(total / N)


def kernel(x, centers, labels):
    nc = _get_nc()
    res = run_bass_kernel_spmd(
        nc, make_in_maps(x, centers, labels), core_ids=list(range(N_CORES))
    )
    return finalize(res.results)


# revision 15
# speedup vs baseline: 2.5913x; 1.3396x over previous
"""CenterLoss Trainium2 kernel (8 NeuronCores, data-parallel over batch).

Math: the reference builds the full [N, C] masked distance matrix, but only
the labeled entry of each row survives the mask, so

    loss = ( sum_i ||x_i - centers[labels_i]||^2  +  N*(C-1)*CLAMP_MIN ) / N

(the second term is the clamp applied to the zeroed-out entries). Expanding
||x_i - c||^2 = ||x_i||^2 - 2 x_i.c + ||c||^2:

    sum_i d_i = sum(x*x) + sum_c n_c ||c_c||^2 - 2 sum_i x_i . c_{l_i}

Band strategy (v5): the host sorts each core's 2048 samples by label, so
each 128-sample tile's labels span < 128 consecutive centers (measured max
span 79 for this problem size). The cross term then only needs a [128, 128]
BAND of the x @ centers^T product per tile — 16 small PE matmuls instead of
a per-sample gather (which is Q7-descriptor-bound at ~1us per 128 rows).
The labeled entry of each band row is selected with an on-device one-hot
(iota == rel) and a fused multiply-reduce (DVE tensor_tensor_reduce).
n_c (the label histogram, metadata derived from labels only) is computed
host-side; ||c||^2 is computed on device by ACT square-accumulate.

Host prep is layout-only: sort/transpose/slice of inputs + label metadata.
All arithmetic on x and centers happens on device. Falls back to the v4
indirect-DMA gather kernel if any tile span exceeds the band width.
"""

import numpy as np

import concourse.bacc as bacc
import concourse.tile as tile
from concourse import bass, mybir
from concourse.bass_utils import run_bass_kernel_spmd

N, C, D = 16384, 1024, 128
N_CORES = 8
NS = N // N_CORES  # 2048 samples per core
P = 128
T = NS // P  # 16 tiles per core
W = 128  # band width
CLAMP_MIN = 1e-12

_cache = {}


# ---------------------------------------------------------------- v5: band
def build_nc_band(n_xchunk=4, n_ttr=4):
    nc = bacc.Bacc()
    xst = nc.declare_dram_parameter("xst", [D, NS], mybir.dt.float32, isOutput=False)
    cb = nc.declare_dram_parameter("cb", [D, T * W], mybir.dt.float32, isOutput=False)
    rel = nc.declare_dram_parameter("rel", [P, T], mybir.dt.float32, isOutput=False)
    cnt = nc.declare_dram_parameter(
        "cnt", [P, C // P], mybir.dt.float32, isOutput=False
    )
    centers = nc.declare_dram_parameter(
        "centers", [C, D], mybir.dt.float32, isOutput=False
    )
    out = nc.declare_dram_parameter("out", [1, 1], mybir.dt.float32, isOutput=True)

    c_t = centers.rearrange("(u p) d -> p u d", p=P)  # [128, 8, 128]
    UC = C // P  # 8

    with tile.TileContext(nc) as tc:
        with (
            tc.tile_pool(name="data", bufs=1) as data,
            tc.tile_pool(name="small", bufs=1) as small,
            tc.tile_pool(name="psum", bufs=7, space="PSUM") as psump,
            tc.tile_pool(name="psumr", bufs=1, space="PSUM") as psumr,
        ):
            x_sb = data.tile([P, NS], mybir.dt.float32)
            cb_sb = data.tile([P, T, W], mybir.dt.float32)
            oh_sb = data.tile([P, T, W], mybir.dt.float32)
            c_sb = data.tile([P, UC, D], mybir.dt.float32)
            rel_sb = small.tile([P, T], mybir.dt.float32)
            cnt_sb = small.tile([P, UC], mybir.dt.float32)
            csq_sb = small.tile([P, UC], mybir.dt.float32)
            iota_i = small.tile([P, W], mybir.dt.int32)
            iota_f = small.tile([P, W], mybir.dt.float32)
            # accumulator columns: [0:n_xchunk] = x^2, [n_xchunk] = cross
            # (needs weight -2), [n_xchunk+1] = n*csq
            n_acc = n_xchunk + 2
            acc = small.tile([P, n_acc], mybir.dt.float32)
            colw = small.tile([1, n_acc], mybir.dt.float32)
            tmp8 = small.tile([P, C // P], mybir.dt.float32)
            ones = small.tile([P, 1], mybir.dt.float32)

            nc.vector.memset(ones[:], 1.0)
            nc.gpsimd.iota(iota_i[:], pattern=[[1, W]], base=0, channel_multiplier=0)
            nc.vector.tensor_copy(out=iota_f[:], in_=iota_i[:])

            nc.sync.dma_start(out=rel_sb[:], in_=rel[:, :])
            nc.sync.dma_start(out=cnt_sb[:], in_=cnt[:, :])
            # one-hot: oh[p, t, w] = (iota[w] == rel[p, t])
            nc.vector.tensor_tensor(
                out=oh_sb[:],
                in0=iota_f[:, None, :].to_broadcast([P, T, W]),
                in1=rel_sb[:, :, None].to_broadcast([P, T, W]),
                op=mybir.AluOpType.is_equal,
            )

            spx = NS // n_xchunk

            nc.sync.dma_start(out=c_sb[:], in_=c_t[:, :, :])
            for j in range(n_xchunk):
                xs = slice(j * spx, (j + 1) * spx)
                nc.sync.dma_start(out=x_sb[:, xs], in_=xst[:, xs])
            for k in range(n_ttr):
                tpb = T // n_ttr
                ts = slice(k * tpb, (k + 1) * tpb)
                nc.sync.dma_start(out=cb_sb[:, ts, :], in_=cb[:, k * tpb * W : (k + 1) * tpb * W].rearrange("p (t w) -> p t w", w=W))
            # band matmuls: dot[s, w] = sum_d x[d, s] * cb[d, w]
            # one PSUM bank per matmul; mask with the one-hot in place
            for t in range(T):
                psum_t = psump.tile([P, W], mybir.dt.float32, tag="band")
                nc.tensor.matmul(
                    out=psum_t[:, :],
                    lhsT=x_sb[:, t * P : (t + 1) * P],
                    rhs=cb_sb[:, t, :],
                    start=True,
                    stop=True,
                )
                nc.vector.tensor_tensor(
                    out=oh_sb[:, t, :],
                    in0=oh_sb[:, t, :],
                    in1=psum_t[:, :],
                    op=mybir.AluOpType.mult,
                )
            nc.vector.reduce_sum(
                out=acc[:, n_xchunk : n_xchunk + 1],
                in_=oh_sb[:],
                axis=mybir.AxisListType.XY,
            )
            # ||x||^2 chunks on ACT
            for j in range(n_xchunk):
                xs = slice(j * spx, (j + 1) * spx)
                nc.scalar.activation(
                    out=x_sb[:, xs],
                    in_=x_sb[:, xs],
                    func=mybir.ActivationFunctionType.Square,
                    accum_out=acc[:, j : j + 1],
                )
            # csq[p, u] = ||centers[u*128+p]||^2 on ACT
            for u in range(UC):
                nc.scalar.activation(
                    out=c_sb[:, u, :],
                    in_=c_sb[:, u, :],
                    func=mybir.ActivationFunctionType.Square,
                    accum_out=csq_sb[:, u : u + 1],
                )
            # acc[last] = sum_u cnt * csq
            nc.vector.tensor_tensor(
                out=tmp8[:],
                in0=cnt_sb[:],
                in1=csq_sb[:],
                op=mybir.AluOpType.mult,
            )
            nc.vector.reduce_sum(
                out=acc[:, n_xchunk + 1 : n_xchunk + 2],
                in_=tmp8[:],
                axis=mybir.AxisListType.X,
            )
            # column weights: +1 everywhere, -2 on the cross column
            nc.vector.memset(colw[:1, :], 1.0)
            nc.vector.memset(colw[:1, n_xchunk : n_xchunk + 1], -2.0)
            psum_r = psumr.tile([1, n_acc], mybir.dt.float32)
            nc.tensor.matmul(
                out=psum_r[:, :], lhsT=ones[:], rhs=acc[:], start=True, stop=True
            )
            wsum = small.tile([1, n_acc], mybir.dt.float32)
            nc.vector.tensor_tensor(
                out=wsum[:1, :],
                in0=colw[:1, :],
                in1=psum_r[:1, :],
                op=mybir.AluOpType.mult,
            )
            res = small.tile([1, 1], mybir.dt.float32)
            nc.vector.reduce_sum(
                out=res[:1, :1], in_=wsum[:1, :], axis=mybir.AxisListType.X
            )
            nc.sync.dma_start(out=out[:, :], in_=res[:1, :1])
    nc.compile()
    return nc


def prep_band_core(x_shard, labels_shard, ct):
    """Host layout prep for one core. Returns in_map or None if a tile span
    exceeds the band width."""
    order = np.argsort(labels_shard, kind="stable")
    ls = labels_shard[order].astype(np.int64)
    bases = np.minimum(ls[:: P][:T], C - W)  # [T]
    rel = ls.reshape(T, P).T - bases[None, :]  # [128, T]
    if rel.min() < 0 or rel.max() >= W:
        return None
    xs = x_shard[order]  # [NS, D]
    cb = np.concatenate([ct[:, b : b + W] for b in bases], axis=1)  # [D, T*W]
    cnt = np.bincount(labels_shard.astype(np.int64), minlength=C).astype(np.float32)
    return {
        "xst": np.ascontiguousarray(xs.T),
        "cb": np.ascontiguousarray(cb),
        "rel": np.ascontiguousarray(rel.astype(np.float32)),
        "cnt": np.ascontiguousarray(cnt.reshape(C // P, P).T),
        "centers": None,  # filled by caller
    }


# ------------------------------------------------- v4: indirect-DMA gather
def build_nc_gather(n_chunk=4, n_xdma=4):
    nc = bacc.Bacc()
    x = nc.declare_dram_parameter("x", [NS, D], mybir.dt.float32, isOutput=False)
    centers = nc.declare_dram_parameter(
        "centers", [C, D], mybir.dt.float32, isOutput=False
    )
    labels = nc.declare_dram_parameter("labels", [P, T], mybir.dt.int32, isOutput=False)
    out = nc.declare_dram_parameter("out", [1, 1], mybir.dt.float32, isOutput=True)

    x_t = x.rearrange("(t p) d -> p t d", p=P)
    tpc = T // n_chunk

    with tile.TileContext(nc) as tc:
        with (
            tc.tile_pool(name="data", bufs=1) as data,
            tc.tile_pool(name="small", bufs=1) as small,
            tc.tile_pool(name="psum", bufs=1, space="PSUM") as psump,
        ):
            x_sb = data.tile([P, T, D], mybir.dt.float32)
            g_sb = data.tile([P, T, D], mybir.dt.float32)
            d_sb = data.tile([P, T, D], mybir.dt.float32)
            i_sb = small.tile([P, T], mybir.dt.int32)
            acc = small.tile([P, n_chunk], mybir.dt.float32)
            ones = small.tile([P, 1], mybir.dt.float32)

            nc.vector.memset(ones[:], 1.0)
            nc.sync.dma_start(out=i_sb[:], in_=labels[:, :])
            tpx = T // n_xdma
            for j in range(n_xdma):
                xs = slice(j * tpx, (j + 1) * tpx)
                nc.sync.dma_start(out=x_sb[:, xs, :], in_=x_t[:, xs, :])
            for t in range(T):
                nc.gpsimd.indirect_dma_start(
                    out=g_sb[:, t, :],
                    out_offset=None,
                    in_=centers[:],
                    in_offset=bass.IndirectOffsetOnAxis(ap=i_sb[:, t : t + 1], axis=0),
                )
            for k in range(n_chunk):
                ts = slice(k * tpc, (k + 1) * tpc)
                nc.vector.tensor_tensor(
                    out=d_sb[:, ts, :],
                    in0=x_sb[:, ts, :],
                    in1=g_sb[:, ts, :],
                    op=mybir.AluOpType.subtract,
                )
                nc.scalar.activation(
                    out=d_sb[:, ts, :],
                    in_=d_sb[:, ts, :],
                    func=mybir.ActivationFunctionType.Square,
                    accum_out=acc[:, k : k + 1],
                )
            psum = psump.tile([1, n_chunk], mybir.dt.float32)
            nc.tensor.matmul(
                out=psum[:, :], lhsT=ones[:], rhs=acc[:], start=True, stop=True
            )
            res = small.tile([1, 1], mybir.dt.float32)
            nc.vector.reduce_sum(
                out=res[:1, :1], in_=psum[:1, :], axis=mybir.AxisListType.X
            )
            nc.sync.dma_start(out=out[:, :], in_=res[:1, :1])
    nc.compile()
    return nc


# ----------------------------------------------------------------- driver
def make_in_maps(x, centers, labels):
    """Returns (in_maps, which) where which is 'band' or 'gather'."""
    x = np.ascontiguousarray(np.asarray(x, dtype=np.float32))
    centers = np.ascontiguousarray(np.asarray(centers, dtype=np.float32))
    labels = np.asarray(labels)
    ct = np.ascontiguousarray(centers.T)
    in_maps = []
    for c in range(N_CORES):
        sl = slice(c * NS, (c + 1) * NS)
        m = prep_band_core(x[sl], labels[sl], ct)
        if m is None:
            break
        m["centers"] = centers
        in_maps.append(m)
    else:
        return in_maps, "band"
    # fallback: indirect gather kernel
    in_maps = []
    for c in range(N_CORES):
        sl = slice(c * NS, (c + 1) * NS)
        in_maps.append(
            {
                "x": x[sl],
                "centers": centers,
                "labels": np.ascontiguousarray(
                    labels[sl].reshape(T, P).T.astype(np.int32)
                ),
            }
        )
    return in_maps, "gather"


def _get_nc(which):
    if which not in _cache:
        _cache[which] = (
            build_nc_band() if which == "band" else build_nc_gather()
        )
    return _cache[which]


def finalize(results):
    total = sum(float(results[c]["out"][0, 0]) for c in range(N_CORES))
    total += N * (C - 1) * CLAMP_MIN
    return np.float32(total / N)


def kernel(x, centers, labels):
    in_maps, which = make_in_maps(x, centers, labels)
    nc = _get_nc(which)
    res = run_bass_kernel_spmd(nc, in_maps, core_ids=list(range(N_CORES)))
    return finalize(res.results)


# revision 17
# speedup vs baseline: 3.0713x; 1.1852x over previous
"""CenterLoss Trainium2 kernel (8 NeuronCores, data-parallel over batch).

Math: the reference builds the full [N, C] masked distance matrix, but only
the labeled entry of each row survives the mask, so

    loss = ( sum_i ||x_i - centers[labels_i]||^2  +  N*(C-1)*CLAMP_MIN ) / N

(the second term is the clamp applied to the zeroed-out entries). Expanding
||x_i - c||^2 = ||x_i||^2 - 2 x_i.c + ||c||^2:

    sum_i d_i = sum(x*x) + sum_c n_c ||c_c||^2 - 2 sum_i x_i . c_{l_i}

Band strategy (v5): the host sorts each core's 2048 samples by label, so
each 128-sample tile's labels span < 128 consecutive centers (measured max
span 79 for this problem size). The cross term then only needs a [128, 128]
BAND of the x @ centers^T product per tile — 16 small PE matmuls instead of
a per-sample gather (which is Q7-descriptor-bound at ~1us per 128 rows).
The labeled entry of each band row is selected with an on-device one-hot
(iota == rel) and a fused multiply-reduce (DVE tensor_tensor_reduce).
n_c (the label histogram, metadata derived from labels only) is computed
host-side; ||c||^2 is computed on device by ACT square-accumulate.

Host prep is layout-only: sort/transpose/slice of inputs + label metadata.
All arithmetic on x and centers happens on device. Falls back to the v4
indirect-DMA gather kernel if any tile span exceeds the band width.
"""

import numpy as np

import concourse.bacc as bacc
import concourse.tile as tile
from concourse import bass, mybir
from concourse.bass_utils import run_bass_kernel_spmd

N, C, D = 16384, 1024, 128
N_CORES = 8
NS = N // N_CORES  # 2048 samples per core
P = 128
T = NS // P  # 16 tiles per core
W = 128  # band width
CLAMP_MIN = 1e-12

_cache = {}


# ---------------------------------------------------------------- v5: band
def build_nc_band(n_xchunk=2, n_mask=4):
    nc = bacc.Bacc()
    xst = nc.declare_dram_parameter("xst", [D, NS], mybir.dt.bfloat16, isOutput=False)
    cb = nc.declare_dram_parameter("cb", [D, T * W], mybir.dt.bfloat16, isOutput=False)
    # small: cols [0:T] = rel, [T:T+8] = cnt
    small_in = nc.declare_dram_parameter(
        "small", [P, T + C // P], mybir.dt.float32, isOutput=False
    )
    centers = nc.declare_dram_parameter(
        "centers", [C, D], mybir.dt.float32, isOutput=False
    )
    out = nc.declare_dram_parameter("out", [1, 1], mybir.dt.float32, isOutput=True)

    c_t = centers.rearrange("(u p) d -> p u d", p=P)  # [128, 8, 128]
    UC = C // P  # 8

    with tile.TileContext(nc) as tc:
        with (
            tc.tile_pool(name="data", bufs=1) as data,
            tc.tile_pool(name="small", bufs=1) as small,
            tc.tile_pool(name="psum", bufs=1, space="PSUM") as psump,
            tc.tile_pool(name="psumr", bufs=1, space="PSUM") as psumr,
        ):
            x_sb = data.tile([P, NS], mybir.dt.bfloat16)
            cb_sb = data.tile([P, T, W], mybir.dt.bfloat16)
            oh_sb = data.tile([P, T, W], mybir.dt.float32)
            c_sb = data.tile([P, UC, D], mybir.dt.float32)
            sm_sb = small.tile([P, T + UC], mybir.dt.float32)
            csq_sb = small.tile([P, UC], mybir.dt.float32)
            iota_i = small.tile([P, W], mybir.dt.int32)
            iota_f = small.tile([P, W], mybir.dt.float32)
            # accumulator columns: [0:n_xchunk] = x^2, [n_xchunk] = cross
            # (weight -2), [n_xchunk+1] = n*csq
            n_acc = n_xchunk + 2
            acc = small.tile([P, n_acc], mybir.dt.float32)
            colw = small.tile([1, n_acc], mybir.dt.float32)
            tmp8 = small.tile([P, UC], mybir.dt.float32)
            ones = small.tile([P, 1], mybir.dt.float32)

            nc.vector.memset(ones[:], 1.0)
            nc.gpsimd.iota(iota_i[:], pattern=[[1, W]], base=0, channel_multiplier=0)
            nc.vector.tensor_copy(out=iota_f[:], in_=iota_i[:])

            nc.scalar.dma_start(out=sm_sb[:], in_=small_in[:, :])
            rel_sb = sm_sb[:, 0:T]
            cnt_sb = sm_sb[:, T : T + UC]
            # one-hot: oh[p, t, w] = (iota[w] == rel[p, t])
            nc.vector.tensor_tensor(
                out=oh_sb[:],
                in0=iota_f[:, None, :].to_broadcast([P, T, W]),
                in1=rel_sb[:, :, None].to_broadcast([P, T, W]),
                op=mybir.AluOpType.is_equal,
            )

            nc.sync.dma_start(out=x_sb[:], in_=xst[:, :])
            nc.sync.dma_start(
                out=cb_sb[:], in_=cb[:, :].rearrange("p (t w) -> p t w", w=W)
            )
            nc.scalar.dma_start(out=c_sb[:], in_=c_t[:, :, :])
            # band matmuls: dot[s, w] = sum_d x[d, s] * cb[d, w]
            psum_big = psump.tile([P, T, W], mybir.dt.float32)
            for t in range(T):
                nc.tensor.matmul(
                    out=psum_big[:, t, :],
                    lhsT=x_sb[:, t * P : (t + 1) * P],
                    rhs=cb_sb[:, t, :],
                    start=True,
                    stop=True,
                )
            # mask band dots with the one-hot (in place), then one big reduce
            tpm = T // n_mask
            for k in range(n_mask):
                ts = slice(k * tpm, (k + 1) * tpm)
                nc.vector.tensor_tensor(
                    out=oh_sb[:, ts, :],
                    in0=oh_sb[:, ts, :],
                    in1=psum_big[:, ts, :],
                    op=mybir.AluOpType.mult,
                )
            nc.vector.reduce_sum(
                out=acc[:, n_xchunk : n_xchunk + 1],
                in_=oh_sb[:],
                axis=mybir.AxisListType.XY,
            )
            # ||x||^2 chunks on ACT (bf16 in, fp32 accumulate)
            spx = NS // n_xchunk
            for j in range(n_xchunk):
                xs = slice(j * spx, (j + 1) * spx)
                nc.scalar.activation(
                    out=x_sb[:, xs],
                    in_=x_sb[:, xs],
                    func=mybir.ActivationFunctionType.Square,
                    accum_out=acc[:, j : j + 1],
                )
            # csq[p, u] = ||centers[u*128+p]||^2 on ACT
            for u in range(UC):
                nc.scalar.activation(
                    out=c_sb[:, u, :],
                    in_=c_sb[:, u, :],
                    func=mybir.ActivationFunctionType.Square,
                    accum_out=csq_sb[:, u : u + 1],
                )
            # acc[last] = sum_u cnt * csq
            nc.vector.tensor_tensor(
                out=tmp8[:],
                in0=cnt_sb[:],
                in1=csq_sb[:],
                op=mybir.AluOpType.mult,
            )
            nc.vector.reduce_sum(
                out=acc[:, n_xchunk + 1 : n_xchunk + 2],
                in_=tmp8[:],
                axis=mybir.AxisListType.X,
            )
            # column weights: +1 everywhere, -2 on the cross column
            nc.vector.memset(colw[:1, :], 1.0)
            nc.vector.memset(colw[:1, n_xchunk : n_xchunk + 1], -2.0)
            psum_r = psumr.tile([1, n_acc], mybir.dt.float32)
            nc.tensor.matmul(
                out=psum_r[:, :], lhsT=ones[:], rhs=acc[:], start=True, stop=True
            )
            wsum = small.tile([1, n_acc], mybir.dt.float32)
            nc.vector.tensor_tensor(
                out=wsum[:1, :],
                in0=colw[:1, :],
                in1=psum_r[:1, :],
                op=mybir.AluOpType.mult,
            )
            res = small.tile([1, 1], mybir.dt.float32)
            nc.vector.reduce_sum(
                out=res[:1, :1], in_=wsum[:1, :], axis=mybir.AxisListType.X
            )
            nc.sync.dma_start(out=out[:, :], in_=res[:1, :1])
    nc.compile()
    return nc


def prep_band_core(x_shard, labels_shard, ct):
    """Host layout prep for one core. Returns in_map or None if a tile span
    exceeds the band width."""
    import ml_dtypes

    order = np.argsort(labels_shard, kind="stable")
    ls = labels_shard[order].astype(np.int64)
    bases = np.minimum(ls[::P][:T], C - W)  # [T]
    rel = ls.reshape(T, P).T - bases[None, :]  # [128, T]
    if rel.min() < 0 or rel.max() >= W:
        return None
    xs = x_shard[order]  # [NS, D]
    cb = np.concatenate([ct[:, b : b + W] for b in bases], axis=1)  # [D, T*W]
    cnt = np.bincount(labels_shard.astype(np.int64), minlength=C).astype(np.float32)
    small = np.concatenate(
        [rel.astype(np.float32), cnt.reshape(C // P, P).T], axis=1
    )
    return {
        "xst": np.ascontiguousarray(xs.T.astype(ml_dtypes.bfloat16)),
        "cb": np.ascontiguousarray(cb.astype(ml_dtypes.bfloat16)),
        "small": np.ascontiguousarray(small),
        "centers": None,  # filled by caller
    }


# ------------------------------------------------- v4: indirect-DMA gather
def build_nc_gather(n_chunk=4, n_xdma=4):
    nc = bacc.Bacc()
    x = nc.declare_dram_parameter("x", [NS, D], mybir.dt.float32, isOutput=False)
    centers = nc.declare_dram_parameter(
        "centers", [C, D], mybir.dt.float32, isOutput=False
    )
    labels = nc.declare_dram_parameter("labels", [P, T], mybir.dt.int32, isOutput=False)
    out = nc.declare_dram_parameter("out", [1, 1], mybir.dt.float32, isOutput=True)

    x_t = x.rearrange("(t p) d -> p t d", p=P)
    tpc = T // n_chunk

    with tile.TileContext(nc) as tc:
        with (
            tc.tile_pool(name="data", bufs=1) as data,
            tc.tile_pool(name="small", bufs=1) as small,
            tc.tile_pool(name="psum", bufs=1, space="PSUM") as psump,
        ):
            x_sb = data.tile([P, T, D], mybir.dt.float32)
            g_sb = data.tile([P, T, D], mybir.dt.float32)
            d_sb = data.tile([P, T, D], mybir.dt.float32)
            i_sb = small.tile([P, T], mybir.dt.int32)
            acc = small.tile([P, n_chunk], mybir.dt.float32)
            ones = small.tile([P, 1], mybir.dt.float32)

            nc.vector.memset(ones[:], 1.0)
            nc.sync.dma_start(out=i_sb[:], in_=labels[:, :])
            tpx = T // n_xdma
            for j in range(n_xdma):
                xs = slice(j * tpx, (j + 1) * tpx)
                nc.sync.dma_start(out=x_sb[:, xs, :], in_=x_t[:, xs, :])
            for t in range(T):
                nc.gpsimd.indirect_dma_start(
                    out=g_sb[:, t, :],
                    out_offset=None,
                    in_=centers[:],
                    in_offset=bass.IndirectOffsetOnAxis(ap=i_sb[:, t : t + 1], axis=0),
                )
            for k in range(n_chunk):
                ts = slice(k * tpc, (k + 1) * tpc)
                nc.vector.tensor_tensor(
                    out=d_sb[:, ts, :],
                    in0=x_sb[:, ts, :],
                    in1=g_sb[:, ts, :],
                    op=mybir.AluOpType.subtract,
                )
                nc.scalar.activation(
                    out=d_sb[:, ts, :],
                    in_=d_sb[:, ts, :],
                    func=mybir.ActivationFunctionType.Square,
                    accum_out=acc[:, k : k + 1],
                )
            psum = psump.tile([1, n_chunk], mybir.dt.float32)
            nc.tensor.matmul(
                out=psum[:, :], lhsT=ones[:], rhs=acc[:], start=True, stop=True
            )
            res = small.tile([1, 1], mybir.dt.float32)
            nc.vector.reduce_sum(
                out=res[:1, :1], in_=psum[:1, :], axis=mybir.AxisListType.X
            )
            nc.sync.dma_start(out=out[:, :], in_=res[:1, :1])
    nc.compile()
    return nc


# ----------------------------------------------------------------- driver
def make_in_maps(x, centers, labels):
    """Returns (in_maps, which) where which is 'band' or 'gather'."""
    x = np.ascontiguousarray(np.asarray(x, dtype=np.float32))
    centers = np.ascontiguousarray(np.asarray(centers, dtype=np.float32))
    labels = np.asarray(labels)
    ct = np.ascontiguousarray(centers.T)
    in_maps = []
    for c in range(N_CORES):
        sl = slice(c * NS, (c + 1) * NS)
        m = prep_band_core(x[sl], labels[sl], ct)
        if m is None:
            break
        m["centers"] = centers
        in_maps.append(m)
    else:
        return in_maps, "band"
    # fallback: indirect gather kernel
    in_maps = []
    for c in range(N_CORES):
        sl = slice(c * NS, (c + 1) * NS)
        in_maps.append(
            {
                "x": x[sl],
                "centers": centers,
                "labels": np.ascontiguousarray(
                    labels[sl].reshape(T, P).T.astype(np.int32)
                ),
            }
        )
    return in_maps, "gather"


def _get_nc(which):
    if which not in _cache:
        _cache[which] = (
            build_nc_band() if which == "band" else build_nc_gather()
        )
    return _cache[which]


def finalize(results):
    total = sum(float(results[c]["out"][0, 0]) for c in range(N_CORES))
    total += N * (C - 1) * CLAMP_MIN
    return np.float32(total / N)


def kernel(x, centers, labels):
    in_maps, which = make_in_maps(x, centers, labels)
    nc = _get_nc(which)
    res = run_bass_kernel_spmd(nc, in_maps, core_ids=list(range(N_CORES)))
    return finalize(res.results)


# revision 19
# speedup vs baseline: 3.3444x; 1.0889x over previous
"""CenterLoss Trainium2 kernel (8 NeuronCores, data-parallel over batch).

Math: the reference builds the full [N, C] masked distance matrix, but only
the labeled entry of each row survives the mask, so

    loss = ( sum_i ||x_i - centers[labels_i]||^2  +  N*(C-1)*CLAMP_MIN ) / N

(the second term is the clamp applied to the zeroed-out entries). Expanding
||x_i - c||^2 = ||x_i||^2 - 2 x_i.c + ||c||^2:

    sum_i d_i = sum(x*x) + sum_c n_c ||c_c||^2 - 2 sum_i x_i . c_{l_i}

Band strategy (v5): the host sorts each core's 2048 samples by label, so
each 128-sample tile's labels span < 128 consecutive centers (measured max
span 79 for this problem size). The cross term then only needs a [128, 128]
BAND of the x @ centers^T product per tile — 16 small PE matmuls instead of
a per-sample gather (which is Q7-descriptor-bound at ~1us per 128 rows).
The labeled entry of each band row is selected with an on-device one-hot
(iota == rel) and a fused multiply-reduce (DVE tensor_tensor_reduce).
n_c (the label histogram, metadata derived from labels only) is computed
host-side; ||c||^2 is computed on device by ACT square-accumulate.

Host prep is layout-only: sort/transpose/slice of inputs + label metadata.
All arithmetic on x and centers happens on device. Falls back to the v4
indirect-DMA gather kernel if any tile span exceeds the band width.
"""

import numpy as np

import concourse.bacc as bacc
import concourse.tile as tile
from concourse import bass, mybir
from concourse.bass_utils import run_bass_kernel_spmd

N, C, D = 16384, 1024, 128
N_CORES = 8
NS = N // N_CORES  # 2048 samples per core
P = 128
T = NS // P  # 16 tiles per core
W = 128  # band width
CLAMP_MIN = 1e-12

_cache = {}


# ---------------------------------------------------------------- v5: band
def build_nc_band(n_xchunk=2, n_mask=4):
    nc = bacc.Bacc()
    xst = nc.declare_dram_parameter("xst", [D, NS], mybir.dt.bfloat16, isOutput=False)
    cb = nc.declare_dram_parameter("cb", [D, T * W], mybir.dt.bfloat16, isOutput=False)
    # small: cols [0:T] = rel, [T:T+8] = cnt
    small_in = nc.declare_dram_parameter(
        "small", [P, T + C // P], mybir.dt.float32, isOutput=False
    )
    centers = nc.declare_dram_parameter(
        "centers", [C, D], mybir.dt.float32, isOutput=False
    )
    out = nc.declare_dram_parameter("out", [1, 1], mybir.dt.float32, isOutput=True)

    c_t = centers.rearrange("(u p) d -> p u d", p=P)  # [128, 8, 128]
    UC = C // P  # 8

    with tile.TileContext(nc) as tc:
        with (
            tc.tile_pool(name="data", bufs=1) as data,
            tc.tile_pool(name="small", bufs=1) as small,
            tc.tile_pool(name="psum", bufs=1, space="PSUM") as psump,
            tc.tile_pool(name="psumr", bufs=1, space="PSUM") as psumr,
        ):
            x_sb = data.tile([P, NS], mybir.dt.bfloat16)
            cb_sb = data.tile([P, T, W], mybir.dt.bfloat16)
            oh_sb = data.tile([P, T, W], mybir.dt.float32)
            c_sb = data.tile([P, UC, D], mybir.dt.float32)
            xsq_sb = data.tile([P, NS], mybir.dt.bfloat16)
            sm_sb = small.tile([P, T + UC], mybir.dt.float32)
            csq_sb = small.tile([P, UC], mybir.dt.float32)
            iota_i = small.tile([P, W], mybir.dt.int32)
            iota_f = small.tile([P, W], mybir.dt.float32)
            # accumulator columns: [0:n_xchunk] = x^2, [n_xchunk:n_xchunk+
            # n_mask] = cross chunks (weight -2), [last] = n*csq
            n_acc = n_xchunk + n_mask + 1
            acc = small.tile([P, n_acc], mybir.dt.float32)
            colw = small.tile([1, n_acc], mybir.dt.float32)
            tmp8 = small.tile([P, UC], mybir.dt.float32)
            ones = small.tile([P, 1], mybir.dt.float32)

            nc.vector.memset(ones[:], 1.0)
            nc.gpsimd.iota(iota_i[:], pattern=[[1, W]], base=0, channel_multiplier=0)
            nc.vector.tensor_copy(out=iota_f[:], in_=iota_i[:])

            nc.scalar.dma_start(out=sm_sb[:], in_=small_in[:, :])
            rel_sb = sm_sb[:, 0:T]
            cnt_sb = sm_sb[:, T : T + UC]
            # split x/cb DMAs so the first matmuls can start earlier
            cb_r = cb[:, :].rearrange("p (t w) -> p t w", w=W)
            h = T // 2
            nc.sync.dma_start(out=x_sb[:, : h * P], in_=xst[:, : h * P])
            nc.sync.dma_start(out=cb_sb[:, :h, :], in_=cb_r[:, :h, :])
            nc.sync.dma_start(out=x_sb[:, h * P :], in_=xst[:, h * P :])
            nc.sync.dma_start(out=cb_sb[:, h:, :], in_=cb_r[:, h:, :])
            nc.scalar.dma_start(out=c_sb[:], in_=c_t[:, :, :])
            # one-hot: oh[p, t, w] = (iota[w] == rel[p, t])
            nc.vector.tensor_tensor(
                out=oh_sb[:],
                in0=iota_f[:, None, :].to_broadcast([P, T, W]),
                in1=rel_sb[:, :, None].to_broadcast([P, T, W]),
                op=mybir.AluOpType.is_equal,
            )
            # band matmuls: dot[s, w] = sum_d x[d, s] * cb[d, w]
            psum_big = psump.tile([P, T, W], mybir.dt.float32)
            tpm = T // n_mask
            for t in range(T):
                nc.tensor.matmul(
                    out=psum_big[:, t, :],
                    lhsT=x_sb[:, t * P : (t + 1) * P],
                    rhs=cb_sb[:, t, :],
                    start=True,
                    stop=True,
                )
                # after each group of tpm matmuls, mask + reduce that chunk
                if t % tpm == tpm - 1:
                    k = t // tpm
                    ts = slice(k * tpm, (k + 1) * tpm)
                    nc.vector.tensor_tensor(
                        out=oh_sb[:, ts, :],
                        in0=oh_sb[:, ts, :],
                        in1=psum_big[:, ts, :],
                        op=mybir.AluOpType.mult,
                    )
                    nc.vector.reduce_sum(
                        out=acc[:, n_xchunk + k : n_xchunk + k + 1],
                        in_=oh_sb[:, ts, :],
                        axis=mybir.AxisListType.XY,
                    )
            # csq[p, u] = ||centers[u*128+p]||^2 on ACT
            for u in range(UC):
                nc.scalar.activation(
                    out=c_sb[:, u, :],
                    in_=c_sb[:, u, :],
                    func=mybir.ActivationFunctionType.Square,
                    accum_out=csq_sb[:, u : u + 1],
                )
            # ||x||^2 chunks on ACT (bf16 in, fp32 accumulate); writes go to
            # a scratch tile so they don't serialize against the matmuls
            spx = NS // n_xchunk
            for j in range(n_xchunk):
                xs = slice(j * spx, (j + 1) * spx)
                nc.scalar.activation(
                    out=xsq_sb[:, xs],
                    in_=x_sb[:, xs],
                    func=mybir.ActivationFunctionType.Square,
                    accum_out=acc[:, j : j + 1],
                )
            # acc[last] = sum_u cnt * csq
            nc.vector.tensor_tensor(
                out=tmp8[:],
                in0=cnt_sb[:],
                in1=csq_sb[:],
                op=mybir.AluOpType.mult,
            )
            nc.vector.reduce_sum(
                out=acc[:, n_xchunk + n_mask : n_xchunk + n_mask + 1],
                in_=tmp8[:],
                axis=mybir.AxisListType.X,
            )
            # column weights: +1 everywhere, -2 on the cross columns
            nc.vector.memset(colw[:1, :], 1.0)
            nc.vector.memset(colw[:1, n_xchunk : n_xchunk + n_mask], -2.0)
            psum_r = psumr.tile([1, n_acc], mybir.dt.float32)
            nc.tensor.matmul(
                out=psum_r[:, :], lhsT=ones[:], rhs=acc[:], start=True, stop=True
            )
            wsum = small.tile([1, n_acc], mybir.dt.float32)
            nc.vector.tensor_tensor(
                out=wsum[:1, :],
                in0=colw[:1, :],
                in1=psum_r[:1, :],
                op=mybir.AluOpType.mult,
            )
            res = small.tile([1, 1], mybir.dt.float32)
            nc.vector.reduce_sum(
                out=res[:1, :1], in_=wsum[:1, :], axis=mybir.AxisListType.X
            )
            nc.sync.dma_start(out=out[:, :], in_=res[:1, :1])
    nc.compile()
    return nc


def prep_band_core(x_shard, labels_shard, ct):
    """Host layout prep for one core. Returns in_map or None if a tile span
    exceeds the band width."""
    import ml_dtypes

    order = np.argsort(labels_shard, kind="stable")
    ls = labels_shard[order].astype(np.int64)
    bases = np.minimum(ls[::P][:T], C - W)  # [T]
    rel = ls.reshape(T, P).T - bases[None, :]  # [128, T]
    if rel.min() < 0 or rel.max() >= W:
        return None
    xs = x_shard[order]  # [NS, D]
    cb = np.concatenate([ct[:, b : b + W] for b in bases], axis=1)  # [D, T*W]
    cnt = np.bincount(labels_shard.astype(np.int64), minlength=C).astype(np.float32)
    small = np.concatenate(
        [rel.astype(np.float32), cnt.reshape(C // P, P).T], axis=1
    )
    return {
        "xst": np.ascontiguousarray(xs.T.astype(ml_dtypes.bfloat16)),
        "cb": np.ascontiguousarray(cb.astype(ml_dtypes.bfloat16)),
        "small": np.ascontiguousarray(small),
        "centers": None,  # filled by caller
    }


# ------------------------------------------------- v4: indirect-DMA gather
def build_nc_gather(n_chunk=4, n_xdma=4):
    nc = bacc.Bacc()
    x = nc.declare_dram_parameter("x", [NS, D], mybir.dt.float32, isOutput=False)
    centers = nc.declare_dram_parameter(
        "centers", [C, D], mybir.dt.float32, isOutput=False
    )
    labels = nc.declare_dram_parameter("labels", [P, T], mybir.dt.int32, isOutput=False)
    out = nc.declare_dram_parameter("out", [1, 1], mybir.dt.float32, isOutput=True)

    x_t = x.rearrange("(t p) d -> p t d", p=P)
    tpc = T // n_chunk

    with tile.TileContext(nc) as tc:
        with (
            tc.tile_pool(name="data", bufs=1) as data,
            tc.tile_pool(name="small", bufs=1) as small,
            tc.tile_pool(name="psum", bufs=1, space="PSUM") as psump,
        ):
            x_sb = data.tile([P, T, D], mybir.dt.float32)
            g_sb = data.tile([P, T, D], mybir.dt.float32)
            d_sb = data.tile([P, T, D], mybir.dt.float32)
            i_sb = small.tile([P, T], mybir.dt.int32)
            acc = small.tile([P, n_chunk], mybir.dt.float32)
            ones = small.tile([P, 1], mybir.dt.float32)

            nc.vector.memset(ones[:], 1.0)
            nc.sync.dma_start(out=i_sb[:], in_=labels[:, :])
            tpx = T // n_xdma
            for j in range(n_xdma):
                xs = slice(j * tpx, (j + 1) * tpx)
                nc.sync.dma_start(out=x_sb[:, xs, :], in_=x_t[:, xs, :])
            for t in range(T):
                nc.gpsimd.indirect_dma_start(
                    out=g_sb[:, t, :],
                    out_offset=None,
                    in_=centers[:],
                    in_offset=bass.IndirectOffsetOnAxis(ap=i_sb[:, t : t + 1], axis=0),
                )
            for k in range(n_chunk):
                ts = slice(k * tpc, (k + 1) * tpc)
                nc.vector.tensor_tensor(
                    out=d_sb[:, ts, :],
                    in0=x_sb[:, ts, :],
                    in1=g_sb[:, ts, :],
                    op=mybir.AluOpType.subtract,
                )
                nc.scalar.activation(
                    out=d_sb[:, ts, :],
                    in_=d_sb[:, ts, :],
                    func=mybir.ActivationFunctionType.Square,
                    accum_out=acc[:, k : k + 1],
                )
            psum = psump.tile([1, n_chunk], mybir.dt.float32)
            nc.tensor.matmul(
                out=psum[:, :], lhsT=ones[:], rhs=acc[:], start=True, stop=True
            )
            res = small.tile([1, 1], mybir.dt.float32)
            nc.vector.reduce_sum(
                out=res[:1, :1], in_=psum[:1, :], axis=mybir.AxisListType.X
            )
            nc.sync.dma_start(out=out[:, :], in_=res[:1, :1])
    nc.compile()
    return nc


# ----------------------------------------------------------------- driver
def make_in_maps(x, centers, labels):
    """Returns (in_maps, which) where which is 'band' or 'gather'."""
    x = np.ascontiguousarray(np.asarray(x, dtype=np.float32))
    centers = np.ascontiguousarray(np.asarray(centers, dtype=np.float32))
    labels = np.asarray(labels)
    ct = np.ascontiguousarray(centers.T)
    in_maps = []
    for c in range(N_CORES):
        sl = slice(c * NS, (c + 1) * NS)
        m = prep_band_core(x[sl], labels[sl], ct)
        if m is None:
            break
        m["centers"] = centers
        in_maps.append(m)
    else:
        return in_maps, "band"
    # fallback: indirect gather kernel
    in_maps = []
    for c in range(N_CORES):
        sl = slice(c * NS, (c + 1) * NS)
        in_maps.append(
            {
                "x": x[sl],
                "centers": centers,
                "labels": np.ascontiguousarray(
                    labels[sl].reshape(T, P).T.astype(np.int32)
                ),
            }
        )
    return in_maps, "gather"


def _get_nc(which):
    if which not in _cache:
        _cache[which] = (
            build_nc_band() if which == "band" else build_nc_gather()
        )
    return _cache[which]


def finalize(results):
    total = sum(float(results[c]["out"][0, 0]) for c in range(N_CORES))
    total += N * (C - 1) * CLAMP_MIN
    return np.float32(total / N)


def kernel(x, centers, labels):
    in_maps, which = make_in_maps(x, centers, labels)
    nc = _get_nc(which)
    res = run_bass_kernel_spmd(nc, in_maps, core_ids=list(range(N_CORES)))
    return finalize(res.results)
